# revision 38
# baseline (speedup 1.0000x reference)
"""MiniCPM3 attention block on 8 Trainium2 NeuronCores — v4.

Sharding: tensor-parallel over heads (5 heads/core); q_b/kv_b column-parallel,
o_proj row-parallel (host sums the 8 partial outputs). The low-rank
a-projection preamble is replicated per core (collectives don't work in this
environment).

v4 over v2 (747828ns harness baseline; ~457us -> ~286us by local paired
marginal timing):
- hidden^T precomputed host-side in bf16 (kills 320 PE transposes, the 20MB
  fp32 hid DMA, and ~100us of DVE copies).
- aw streamed per 128-col m-chunk (fast PE start, 10KB SBUF instead of 46KB);
  startup DMAs ordered so the first matmul's operands land first.
- preamble for token slices 2,3 is a generator interleaved into attention
  chunk 0 so its PE matmuls fill the exp-latency gaps (and ACT exp of chunk 0
  overlaps preamble PE work); o_proj of chunk 0 likewise interleaves into
  attention chunk 1. Scores are software-pipelined against exp.
- rmsnorm scale for q folded into the rope-multiplier rows (rmfq) instead of
  scaling lat_q; stats (Sqrt) emitted at phase boundaries to limit ACT
  activation-table reloads.
- PSUM held to <= 8 banks per phase by pool lifetime cycling; all matmul
  outputs stay within single PSUM banks (HW requirement).
"""

import numpy as np

import concourse.bass as bass
import concourse.tile as tile
import concourse.mybir as mybir
from concourse import bacc
from concourse.bass_utils import run_bass_kernel_spmd

# ---- problem constants (hardcoded per spec) ----
T = 2048
HIDDEN = 2560
N_HEADS = 40
D_NOPE = 64
D_ROPE = 32
D_QK = 96
D_V = 64
Q_RANK = 768
KV_RANK = 256
ROPE_THETA = 10000.0
EPS = 1e-6
SCALE = float(D_QK) ** -0.5

NCORES = 8
HPC = N_HEADS // NCORES  # 5
P = 128
D2 = D_ROPE // 2  # 16

F32 = mybir.dt.float32
BF16 = mybir.dt.bfloat16
AF = mybir.ActivationFunctionType
ALU = mybir.AluOpType

KC_HID = HIDDEN // P          # 20
KC_Q = Q_RANK // P            # 6
KC_KV = KV_RANK // P          # 2
NMC = 9                       # preamble m-chunks: q0..5, kv0..1, pe
NT = T // P                   # 16
NSL = 4                       # preamble token slices
SLT = T // NSL                # 512
CHW = 1024                    # attention tq chunk width
NCH = T // CHW                # 2
JPC = CHW // P                # 8 k-blocks per chunk

LAST_RESULT = None
ts = bass.ts
ds = bass.ds

# m-chunk list: (kind, index-within-kind)
MLIST = [("q", i) for i in range(KC_Q)] + [("kv", i) for i in range(KC_KV)]
MLIST.append(("pe", 0))


def _pull(gen, n):
    if gen is None:
        return
    for _ in range(n):
        try:
            next(gen)
        except StopIteration:
            return


def _body(nc, tc, d, dbg=False, phase=4):
    small = tc.alloc_tile_pool(name="small", bufs=1)
    # mask[i, j] = 1 if i <= j else 0 (tk row i <= tq col j in diag block)
    maskf = small.tile([P, P], F32)
    nc.gpsimd.memset(maskf[:], 1.0)
    nc.gpsimd.affine_select(
        out=maskf[:], in_=maskf[:], compare_op=ALU.is_ge,
        fill=0.0, base=0, pattern=[[1, P]], channel_multiplier=-1,
    )
    mask = small.tile([P, P], BF16)
    nc.vector.tensor_copy(mask[:], maskf[:])
    ones_b = small.tile([P, 1], BF16)
    nc.gpsimd.memset(ones_b[:], 1.0)
    eps_c = small.tile([1, 1], F32)
    nc.gpsimd.memset(eps_c[:], EPS)

    # rope multiplier rows: 0:64 ones, 64:96 [c;c], 96:128 [s;s]
    rmf = small.tile([P, T], BF16)
    rmfq = small.tile([P, T], BF16)      # rmf * s_q(t), filled per chunk
    rrq_sb = small.tile([1, T], BF16)    # s_q(t) per token
    madt = small.tile([P, D_ROPE], BF16)  # rows 64:128 = [I32; I32]
    kroT = small.tile([D_ROPE, T], BF16)  # roped shared k_pe

    # (aw chunk 0 is emitted first on the SP queue, inside pre_pair)

    latp = tc.alloc_tile_pool(name="latp", bufs=1)
    lat_q = latp.tile([P, KC_Q, T], BF16)
    lat_kv = latp.tile([P, KC_KV, T], BF16)
    lat_pe = latp.tile([P, T], BF16)  # rows 64:128: pe*cos / sw*sin

    wqkv = tc.alloc_tile_pool(name="wqkv", bufs=1, side="right")
    qb = wqkv.tile([P, KC_Q, HPC * P], BF16)
    kvb = wqkv.tile([P, KC_KV, HPC * P], BF16)

    awp = tc.alloc_tile_pool(name="awp", bufs=2)
    hidp = tc.alloc_tile_pool(name="hidp", bufs=2)
    x2p = tc.alloc_tile_pool(name="x2p", bufs=2)
    nrmp = tc.alloc_tile_pool(name="nrmp", bufs=1, side="right")
    ps_a = tc.alloc_tile_pool(name="ps_a", bufs=2, space="PSUM")
    ps_sq = tc.alloc_tile_pool(name="ps_sq", bufs=2, space="PSUM")

    sqt_store = {}

    def pre_pair(sa, sb, first=False):
        """Generator: a-proj for token slices (sa, sb), chunk-major.
        Yields at ~1us PE quanta. Stats are emitted by pre_stats (kept out
        of the attention exp hot loop to avoid ACT table reloads)."""
        slices = (sa, sb)
        # DMA emission order matters: the first matmul needs aw chunk 0 and
        # the first k-quarter of hidT[sa]; everything else queues behind.
        hidT = {}
        ha = hidp.tile([P, KC_HID, SLT], BF16, tag="hidT", name=f"hidT{sa}")
        hidT[sa] = ha
        awt = []
        a = awp.tile([P, KC_HID, P], BF16, tag="aw")
        if first:
            nc.scalar.dma_start(ha[:, ts(0, 5)], d["hidT"][:, sa, ts(0, 5)])
            nc.sync.dma_start(a[:], d["aw"][:, 0])
            for kq in range(1, 4):
                nc.scalar.dma_start(
                    ha[:, ts(kq, 5)], d["hidT"][:, sa, ts(kq, 5)])
        else:
            nc.scalar.dma_start(ha[:], d["hidT"][:, sa])
            nc.sync.dma_start(a[:], d["aw"][:, 0])
        awt.append(a)
        hb = hidp.tile([P, KC_HID, SLT], BF16, tag="hidT", name=f"hidT{sb}")
        if first:
            for kq in range(4):
                nc.scalar.dma_start(
                    hb[:, ts(kq, 5)], d["hidT"][:, sb, ts(kq, 5)])
        else:
            nc.scalar.dma_start(hb[:], d["hidT"][:, sb])
        hidT[sb] = hb
        if first:
            # constants queue behind the critical aw chunk 0
            nc.sync.dma_start(rmf[:], d["ropemul"])
            nc.sync.dma_start(madt[D_NOPE:P, :], d["madd"])
        sqt = {}
        for s in slices:
            sqt[s] = ps_sq.tile([33, SLT], F32, tag="sqt", name=f"sqt{s}")
            sqt_store[s] = sqt[s]

        def do_chunk(w, kind, mi, s):
            sl = ts(s, SLT)
            pm = ps_a.tile([P, SLT], F32, tag="pm")
            if kind == "pe":
                pmv = pm[D_NOPE:P]
                wv = w[:, :, 0:D_NOPE]
            else:
                pmv = pm[:]
                wv = w
            for k in range(KC_HID):
                nc.tensor.matmul(
                    pmv, wv[:, k], hidT[s][:, k],
                    start=(k == 0), stop=(k == KC_HID - 1),
                )
                if k % 5 == 4 and k != KC_HID - 1:
                    yield
            if kind == "pe":
                nc.vector.tensor_mul(
                    lat_pe[D_NOPE:P, sl], pmv, rmf[D_NOPE:P, sl])
            else:
                x2 = x2p.tile([P, SLT], BF16, tag="x2")
                nc.scalar.activation(x2[:], pm[:], AF.Square)
                row = 0 if kind == "q" else 32
                kc = KC_Q if kind == "q" else KC_KV
                nc.tensor.matmul(
                    sqt[s][row:row + 1, :], ones_b[:], x2[:],
                    start=(mi == 0), stop=(mi == kc - 1),
                )
                dst = lat_q[:, mi, sl] if kind == "q" else lat_kv[:, mi, sl]
                nc.vector.tensor_copy(dst, pm[:])
            yield

        def fetch_aw(m):
            a = awp.tile([P, KC_HID, P], BF16, tag="aw")
            nc.sync.dma_start(a[:], d["aw"][:, m])
            return a

        # chunk-major: each aw chunk serves both slices
        cur = awt[0]
        for m, (kind, mi) in enumerate(MLIST):
            nxt = fetch_aw(m + 1) if m + 1 < NMC else None
            for s in slices:
                yield from do_chunk(cur, kind, mi, s)
            if nxt is not None:
                cur = nxt
    def pre_stats(slices):
        """rmsnorm stats (ACT Sqrt + DVE reciprocal) + lat_kv scaling.
        Called at phase boundaries so the sqrt table loads never land inside
        the attention exp loop."""
        for s in slices:
            sl = ts(s, SLT)
            sqt = sqt_store[s]
            rrf = nrmp.tile([1, SLT], F32, tag="rrf")
            nc.scalar.activation(rrf[:], sqt[0:1, :], AF.Sqrt,
                                 bias=eps_c[:], scale=1.0 / Q_RANK)
            nc.vector.reciprocal(rrf[:], rrf[:])
            nc.vector.tensor_copy(rrq_sb[:, sl], rrf[:])
            rrk = nrmp.tile([1, SLT], F32, tag="rrk")
            nc.scalar.activation(rrk[:], sqt[32:33, :], AF.Sqrt,
                                 bias=eps_c[:], scale=1.0 / KV_RANK)
            nc.vector.reciprocal(rrk[:], rrk[:])
            rrkv = nrmp.tile([1, SLT], BF16, tag="rrkv")
            nc.vector.tensor_copy(rrkv[:], rrk[:])
            bq = nrmp.tile([P, SLT], BF16, tag="bq")
            nc.gpsimd.partition_broadcast(bq[:], rrkv[:])
            for k in range(KC_KV):
                nc.vector.tensor_mul(lat_kv[:, k, sl], lat_kv[:, k, sl], bq[:])

    # ============ phase A: slices 0,1 ============
    ga = pre_pair(0, 1, first=True)
    _pull(ga, 2)
    # per-head weights: queued on the scalar DGE behind the first hidT slices
    nc.scalar.dma_start(qb[:], d["qb"].rearrange("(k p) m -> p k m", p=P))
    nc.scalar.dma_start(kvb[:], d["kvb"].rearrange("(k p) m -> p k m", p=P))
    _pull(ga, 100000)
    pre_stats((0, 1))

    if dbg:
        nc.sync.dma_start(d["dbg_latq"], lat_q[:])
        nc.sync.dma_start(d["dbg_latkv"], lat_kv[:])
        nc.sync.dma_start(d["dbg_latpe"], lat_pe[:])
        nc.sync.dma_start(d["dbg_rrq"], rrq_sb[:])
    if phase < 2:
        nc.sync.dma_start(d["out"][0:P, 0:T], lat_q[:, 0])
        return

    # ============ qkv pools ============
    qkvp = tc.alloc_tile_pool(name="qkvp", bufs=1, side="right")
    qT = qkvp.tile([P, HPC, T], BF16)
    kT = qkvp.tile([P, HPC, T], BF16)
    V = qkvp.tile([P, NT, HPC, D_V + 1], BF16)
    nc.vector.tensor_copy(
        V[:, :, :, 0:1],
        ones_b[:, :, None, None].to_broadcast([P, NT, HPC, 1]))

    kvr = kvb.rearrange("p k (h d) -> p k h d", h=HPC)

    def qkv_chunk(c, ps_b):
        """kroT, kT, V, rmfq, qT for attention chunk c (cols c*CHW ..).
        All PSUM from one [P, CHW] tag so the pool stays at 4 banks."""
        cl = ts(c, CHW)
        # roped k_pe = [I32; I32]^T @ lat_pe[64:128] (cross-partition add)
        kps = ps_b.tile([P, CHW], F32, tag="b")
        for half in range(2):
            cols = ds(c * CHW + half * 512, 512)
            nc.tensor.matmul(kps[0:D_ROPE, ds(half * 512, 512)],
                             madt[D_NOPE:P, :], lat_pe[D_NOPE:P, cols],
                             start=True, stop=True)
        nc.vector.tensor_copy(kroT[:, cl], kps[0:D_ROPE, :])
        # kT nope rows (evacs alternate ACT/DVE to avoid single-engine stall)
        for h in range(HPC):
            kps = ps_b.tile([P, CHW], F32, tag="b")
            for k in range(KC_KV):
                for half in range(2):
                    hsl = ds(half * 512, 512)
                    nc.tensor.matmul(
                        kps[0:D_NOPE, hsl], kvr[:, k, h, 0:D_NOPE],
                        lat_kv[:, k, ds(c * CHW + half * 512, 512)],
                        start=(k == 0), stop=(k == KC_KV - 1),
                    )
            if h % 2 == 0:
                nc.scalar.copy(kT[0:D_NOPE, h, cl], kps[0:D_NOPE, :])
            else:
                nc.vector.tensor_copy(kT[0:D_NOPE, h, cl], kps[0:D_NOPE, :])
        # duplicated roped rows (q-side rope add happens inside the scores
        # matmul by contracting over 128 rows)
        for h in range(HPC):
            nc.sync.dma_start(kT[D_NOPE:D_NOPE + D_ROPE, h, cl], kroT[:, cl])
            nc.sync.dma_start(kT[D_NOPE + D_ROPE:P, h, cl], kroT[:, cl])
        # V: 2 t-tiles per PSUM tile at 512-aligned offsets (a matmul output
        # must not cross a PSUM bank boundary)
        tt0 = c * JPC
        for g in range(JPC // 2):
            tts = (tt0 + 2 * g, tt0 + 2 * g + 1)
            vps = ps_b.tile([P, CHW], F32, tag="b")
            for i, tt in enumerate(tts):
                for k in range(KC_KV):
                    nc.tensor.matmul(
                        vps[:, ds(i * 512, HPC * D_V)],
                        lat_kv[:, k, ts(tt, P)], kvr[:, k, :, D_NOPE:],
                        start=(k == 0), stop=(k == KC_KV - 1),
                    )
            for i, tt in enumerate(tts):
                nc.vector.tensor_copy(
                    V[:, tt, :, 1:D_V + 1],
                    vps[:, ds(i * 512, HPC * D_V)].rearrange(
                        "p (h d) -> p h d", h=HPC))
        # rmfq for this chunk
        rq = nrmp.tile([P, CHW], BF16, tag="rqb")
        nc.gpsimd.partition_broadcast(rq[:], rrq_sb[:, cl])
        nc.vector.tensor_mul(rmfq[:, cl], rmf[:, cl], rq[:])
        # qT with fused rope multiplier (and s_q) on evacuation
        for h in range(HPC):
            qps = ps_b.tile([P, CHW], F32, tag="b")
            for k in range(KC_Q):
                for half in range(2):
                    hsl = ds(half * 512, 512)
                    nc.tensor.matmul(
                        qps[:, hsl], qb[:, k, ts(h, P)],
                        lat_q[:, k, ds(c * CHW + half * 512, 512)],
                        start=(k == 0), stop=(k == KC_Q - 1),
                    )
            nc.vector.tensor_mul(qT[:, h, cl], qps[:], rmfq[:, cl])

    # ============ phase B: qkv chunk 0 ============
    ps_b = tc.alloc_tile_pool(name="ps_b", bufs=2, space="PSUM")
    qkv_chunk(0, ps_b)
    ps_b.release()

    # ============ attention ============
    attnp = tc.alloc_tile_pool(name="attnp", bufs=1, side="right")
    attnT = attnp.tile([P, 3, T], BF16)
    nc.gpsimd.memset(attnT[D_NOPE:P, 2], 0.0)

    exp_p = tc.alloc_tile_pool(name="exp_p", bufs=4, side="right")
    ps_sc = tc.alloc_tile_pool(name="ps_sc", bufs=1, space="PSUM", side="right")
    ps_pv = tc.alloc_tile_pool(name="ps_pv", bufs=1, space="PSUM", side="right")

    def attn_chunk(c, filler, sc_pools, per_j_pulls, split_last=False):
        njt = JPC * (c + 1)
        for h in range(HPC):
            pv = ps_pv.tile([D_V + 1, CHW], F32, tag="pv")
            for j in range(njt):
                d0 = max(0, P * (j - JPC * c))
                sps = sc_pools[j % len(sc_pools)].tile(
                    [P, CHW], F32, tag="sps")
                for half in range(2):
                    lo = half * 512
                    hi = lo + 512
                    if hi <= d0:
                        continue
                    b0 = max(d0, lo)
                    nc.tensor.matmul(
                        sps[:, ds(b0, hi - b0)], kT[:, h, ts(j, P)],
                        qT[:, h, ds(c * CHW + b0, hi - b0)],
                        start=True, stop=True,
                    )
                _pull(filler, per_j_pulls)
                ex = exp_p.tile([P, CHW], BF16, tag="ex")
                nc.scalar.activation(ex[:, d0:], sps[:, d0:], AF.Exp)
                if j >= JPC * c:
                    nc.vector.tensor_mul(
                        ex[:, ds(d0, P)], ex[:, ds(d0, P)], mask[:])
                # last j writing cols [0:512) is JPC*c+3 (d0 reaches 512 after)
                last_j = (JPC * c + 3, njt - 1)
                for half in range(2):
                    lo = half * 512
                    hi = lo + 512
                    if hi <= d0:
                        continue
                    b0 = max(d0, lo)
                    nc.tensor.matmul(
                        pv[:, ds(b0, hi - b0)], V[:, j, h],
                        ex[:, ds(b0, hi - b0)],
                        start=(j == 0), stop=(j == last_j[half]),
                    )
            # normalize: denominator is pv row 0 (V ones column is first);
            # partition_broadcast reads partition 0 only. pv is copied out
            # first so its PSUM bank frees early for the next head. For the
            # last head of the last chunk, go in 512-col halves so the
            # trailing o_proj unblocks sooner.
            halves = ((0, 512), (512, 512)) if (split_last and h == HPC - 1) \
                else ((0, CHW),)
            pt = nrmp.tile([D_V + 1, CHW], F32, tag="ptmp")
            for off, w_ in halves:
                hv = ds(off, w_)
                nc.vector.tensor_copy(pt[:, hv], pv[:, hv])
                nc.vector.reciprocal(pt[0:1, hv], pt[0:1, hv])
                bcs = nrmp.tile([D_V + 1, CHW], F32, tag="bcs")
                nc.gpsimd.partition_broadcast(bcs[:, hv], pt[0:1, hv])
                atmp = nrmp.tile([D_V + 1, CHW], BF16, tag="atmp")
                nc.vector.tensor_mul(atmp[:, hv], pt[:, hv], bcs[:, hv])
                nc.sync.dma_start(
                    attnT[(h % 2) * D_V:(h % 2 + 1) * D_V, h // 2,
                          ds(c * CHW + off, w_)],
                    atmp[1:, hv])
            _pull(filler, 2)

    # ============ phase C: attention c0 + preamble s2,s3 ============
    if phase >= 3:
        g23 = pre_pair(2, 3)
        attn_chunk(0, g23, [ps_sc], 2)
        _pull(g23, 1000)  # drain
    else:
        _pull(pre_pair(2, 3), 1000)
    pre_stats((2, 3))

    x2p.release()
    hidp.release()
    awp.release()
    ps_sq.release()
    ps_a.release()

    if dbg:
        nc.sync.dma_start(d["dbg_qT"], qT[:])
        nc.sync.dma_start(d["dbg_kT"], kT[:])
        nc.sync.dma_start(d["dbg_V"], V[:])

    if phase < 3:
        nc.sync.dma_start(d["out"][0:P, 0:T], qT[:, 0])
        return

    # ============ phase D: qkv chunk 1 ============
    owp = tc.alloc_tile_pool(name="owp", bufs=1, side="right")
    ow = owp.tile([P, 3, HIDDEN], BF16)
    nc.sync.dma_start(ow[:], d["ow"])

    ps_b2 = tc.alloc_tile_pool(name="ps_b2", bufs=2, space="PSUM", side="right")
    qkv_chunk(1, ps_b2)
    ps_b2.release()
    latp.release()

    outp = tc.alloc_tile_pool(name="outp", bufs=3, side="right")
    ps_o = tc.alloc_tile_pool(name="ps_o", bufs=2, space="PSUM", side="right")
    ps_sc2 = tc.alloc_tile_pool(name="ps_sc2", bufs=1, space="PSUM", side="right")

    def oproj_gen(c, tail=False):
        """o_proj for attention chunk c, one yield per 512-col psum group."""
        for piece in range(JPC):
            t = c * JPC + piece
            ob = outp.tile([P, HIDDEN], BF16, tag="ob")
            for n in range(HIDDEN // 512):
                ops = ps_o.tile([P, 512], F32, tag="ops")
                for kc in range(3):
                    nc.tensor.matmul(
                        ops[:], attnT[:, kc, ts(t, P)], ow[:, kc, ts(n, 512)],
                        start=(kc == 0), stop=(kc == 2),
                    )
                if tail and (t + n) % 2 == 1:
                    nc.scalar.copy(ob[:, ts(n, 512)], ops[:])
                else:
                    nc.vector.tensor_copy(ob[:, ts(n, 512)], ops[:])
                yield
            nc.sync.dma_start(d["out"][ts(t, P), :], ob[:])
            yield

    # ============ phase E: attention c1 + o_proj c0 ============
    if phase >= 4:
        g_o0 = oproj_gen(0)
        attn_chunk(1, g_o0, [ps_sc, ps_sc2], 1, split_last=True)
        _pull(g_o0, 1000)
        # ============ phase F: o_proj c1 ============
        for _ in oproj_gen(1, tail=True):
            pass
    else:
        attn_chunk(1, None, [ps_sc, ps_sc2], 0)

    if dbg:
        nc.sync.dma_start(d["dbg_attnT"], attnT[:])
    if phase < 4:
        nc.sync.dma_start(d["out"][0:P, 0:T], attnT[:, 0])

    ps_sc2.release()
    ps_o.release()
    ps_b2_dummy = None
    ps_pv.release()
    ps_sc.release()
    outp.release()
    owp.release()
    exp_p.release()
    attnp.release()
    qkvp.release()
    nrmp.release()
    wqkv.release()
    small.release()


def _build(dbg=False, repeat=1, phase=4):
    nc = bacc.Bacc("TRN2", target_bir_lowering=False, debug=False,
                   num_devices=NCORES)
    d = {
        "hidT": nc.dram_tensor(
            "hidT", [P, NSL, KC_HID, SLT], BF16, kind="ExternalInput").ap(),
        "aw": nc.dram_tensor(
            "aw", [P, NMC, KC_HID, P], BF16, kind="ExternalInput").ap(),
        "qb": nc.dram_tensor("qb", [Q_RANK, HPC * P], BF16, kind="ExternalInput").ap(),
        "kvb": nc.dram_tensor("kvb", [KV_RANK, HPC * P], BF16, kind="ExternalInput").ap(),
        "ow": nc.dram_tensor("ow", [P, 3, HIDDEN], BF16, kind="ExternalInput").ap(),
        "ropemul": nc.dram_tensor("ropemul", [P, T], BF16, kind="ExternalInput").ap(),
        "madd": nc.dram_tensor("madd", [D_NOPE, D_ROPE], BF16, kind="ExternalInput").ap(),
        "out": nc.dram_tensor("out", [T, HIDDEN], BF16, kind="ExternalOutput").ap(),
    }
    if dbg:
        for nm, shp, dt in (
            ("dbg_latq", [P, KC_Q, T], BF16), ("dbg_latkv", [P, KC_KV, T], BF16),
            ("dbg_latpe", [P, T], BF16), ("dbg_rrq", [1, T], BF16),
            ("dbg_qT", [P, HPC, T], BF16),
            ("dbg_kT", [P, HPC, T], BF16), ("dbg_V", [P, NT, HPC, D_V + 1], BF16),
            ("dbg_attnT", [P, 3, T], BF16),
        ):
            d[nm] = nc.dram_tensor(nm, shp, dt, kind="ExternalOutput").ap()
    with tile.TileContext(nc) as tc:
        for r in range(repeat):
            _body(nc, tc, d, dbg=dbg, phase=phase)
    nc.compile()
    return nc


def _bf16(x):
    import ml_dtypes
    return np.ascontiguousarray(
        np.asarray(x, np.float32).astype(ml_dtypes.bfloat16))


def _swap_neg(w):
    """Columns [-x2; x1] for neox rope, acting on the last axis of size 32."""
    return np.concatenate([-w[..., D2:], w[..., :D2]], axis=-1)


def make_in_maps(positions, hidden_states, q_a_w, q_a_ln, q_b_w, kv_a_w,
                 kv_a_ln, kv_b_w, o_w):
    pos = np.asarray(positions)
    hid = np.asarray(hidden_states, dtype=np.float32)
    q_a_w = np.asarray(q_a_w, np.float32)
    q_a_ln = np.asarray(q_a_ln, np.float32)
    q_b_w = np.asarray(q_b_w, np.float32)
    kv_a_w = np.asarray(kv_a_w, np.float32)
    kv_a_ln = np.asarray(kv_a_ln, np.float32)
    kv_b_w = np.asarray(kv_b_w, np.float32)
    o_w = np.asarray(o_w, np.float32)

    # hidT[p, s, k, tt] = hid[s*SLT + tt, k*P + p]
    hidT = hid.reshape(NSL, SLT, KC_HID, P).transpose(3, 0, 2, 1)

    # aw m-chunks: q0..5, kv0..1, [pe|sw|zeros]
    pe_w = kv_a_w[:, KV_RANK:]                       # [HIDDEN, 32]
    aw_full = np.concatenate(
        [q_a_w, kv_a_w[:, :KV_RANK], pe_w, _swap_neg(pe_w),
         np.zeros((HIDDEN, P - 2 * D_ROPE), np.float32)], axis=1)  # [H, 1152]
    aw_r = aw_full.reshape(KC_HID, P, NMC, P).transpose(1, 2, 0, 3)

    # qb: per head [nope64 | pe32 | sw32], ln & SCALE folded
    qbv = (q_a_ln[:, None] * q_b_w * SCALE).reshape(Q_RANK, N_HEADS, D_QK)
    qb_ext = np.concatenate(
        [qbv[:, :, :D_NOPE], qbv[:, :, D_NOPE:], _swap_neg(qbv[:, :, D_NOPE:])],
        axis=2)                                      # [Q_RANK, 40, 128]

    # kvb: per head [k_nope | v], ln folded
    kvbv = (kv_a_ln[:, None] * kv_b_w).reshape(KV_RANK, N_HEADS, D_NOPE + D_V)

    # ropemul rows: 0:64 ones; 64:96 [c;c]; 96:128 [s;s]
    inv_freq = 1.0 / (ROPE_THETA ** (np.arange(0, D_ROPE, 2, np.float32) / D_ROPE))
    freqs = pos.astype(np.float32)[:, None] * inv_freq[None, :]   # [T, 16]
    cosv = np.cos(freqs).T                                        # [16, T]
    sinv = np.sin(freqs).T
    ropemul = np.concatenate(
        [np.ones((D_NOPE, T), np.float32), cosv, cosv, sinv, sinv], axis=0)

    in_maps = []
    for c in range(NCORES):
        h0 = c * HPC
        owc = o_w.reshape(N_HEADS, D_V, HIDDEN)[h0:h0 + HPC]      # [5, 64, H]
        ow3 = np.zeros((3, P, HIDDEN), np.float32)
        ow3[0] = owc[0:2].reshape(P, HIDDEN)
        ow3[1] = owc[2:4].reshape(P, HIDDEN)
        ow3[2, :D_V] = owc[4]
        in_maps.append({
            "hidT": _bf16(hidT),
            "aw": _bf16(aw_r),
            "qb": _bf16(qb_ext[:, h0:h0 + HPC].reshape(Q_RANK, HPC * P)),
            "kvb": _bf16(kvbv[:, h0:h0 + HPC].reshape(KV_RANK, HPC * P)),
            "ow": _bf16(ow3.transpose(1, 0, 2)),
            "ropemul": _bf16(ropemul),
            "madd": _bf16(np.concatenate(
                [np.eye(D_ROPE, dtype=np.float32)] * 2, axis=0)),
        })
    return in_maps


def kernel(positions, hidden_states, q_a_w, q_a_ln, q_b_w, kv_a_w, kv_a_ln,
           kv_b_w, o_w, trace=False):
    global LAST_RESULT
    in_maps = make_in_maps(positions, hidden_states, q_a_w, q_a_ln, q_b_w,
                           kv_a_w, kv_a_ln, kv_b_w, o_w)
    nc = _build()
    res = run_bass_kernel_spmd(nc, in_maps, core_ids=list(range(NCORES)),
                               trace=trace)
    LAST_RESULT = res
    acc = np.zeros((T, HIDDEN), np.float64)
    for c in range(NCORES):
        acc += np.asarray(res.results[c]["out"], np.float64)
    return acc.astype(np.float32)
